# revision 25
# baseline (speedup 1.0000x reference)
"""Trainium2 Bass kernel for nn_MultiHeadAttention (B=4, S=2048, D=512, H=8).

Sharding: tensor-parallel over heads — core c owns head c (Dh=64).
Each core computes q/k/v projections for its head slice (full x replicated,
host-pre-transposed to x^T in bf16), attention for its head over all 4
batches, and the partial out-projection O_c @ Wo[c]; the host sums the 8
partials (the TP all-reduce done at gather time) and adds the biases that
commute with that reduction (bo, bv@Wo).

Engine plan (engines execute their programs in order, so emission order IS
the software pipeline):
  - PE: projections (W-stationary bf16), V^T->V PE-transposes, row-tiled
    S^T pairs (dh=64 contraction on 64x128 array halves), AV with a ones
    column (softmax denominators ride in O^T_aug row 64), out-projection.
  - ACT: exclusively exp(S/8) — it is the pacing engine (~143us floor);
    anything else queued on ACT delays the next attention phase.
  - DVE: all PSUM evacuations + the final normalize (reciprocal of the
    denominator extracted per token partition by an e-column matmul).
  - GPSIMD: bulk x^T loads;  SP: staging shifts + output stores.
Batches are paired [even; odd] on SBUF partition halves; prep of pair 1 is
emitted interleaved with attention of pair 0, and both out-projections are
emitted interleaved with attention of pair 1.
"""
import numpy as np

import concourse.bass as bass
import concourse.mybir as mybir
import concourse.tile as tile
from concourse import bacc
from concourse.bass_utils import run_bass_kernel_spmd

B, S, D = 4, 2048, 512
H, DH = 8, 64
NCORES = 8
F32 = mybir.dt.float32
F32R = mybir.dt.float32r
BF16 = mybir.dt.bfloat16
AF = mybir.ActivationFunctionType

NKT = S // 128          # 16 token tiles per batch
NQB = S // 512          # 4 512-blocks per batch
NCH = D // 128          # 4 dm chunks

_NC_CACHE = {}


def build_kernel():
    nc = bacc.Bacc("TRN2", target_bir_lowering=False, debug=False)

    xT = nc.dram_tensor("xT", [B, D, S], BF16, kind="ExternalInput")
    wq = nc.dram_tensor("wq", [D, DH], BF16, kind="ExternalInput")
    wk = nc.dram_tensor("wk", [D, DH], BF16, kind="ExternalInput")
    wv = nc.dram_tensor("wv", [D, DH], BF16, kind="ExternalInput")
    wo_aug = nc.dram_tensor("wo_aug", [DH + 2, D + 2], BF16, kind="ExternalInput")
    bq = nc.dram_tensor("bq", [DH, 1], F32, kind="ExternalInput")
    bk = nc.dram_tensor("bk", [DH, 1], F32, kind="ExternalInput")
    idin = nc.dram_tensor("idin", [128, 128], BF16, kind="ExternalInput")
    onesin = nc.dram_tensor("onesin", [128, 16, 2], BF16, kind="ExternalInput")
    out = nc.dram_tensor("out", [B * S, D], F32, kind="ExternalOutput")

    with tile.TileContext(nc) as tc:
        with (
            tc.tile_pool(name="consts", bufs=1) as consts,
            tc.tile_pool(name="xtp", bufs=16) as xtp,
            tc.tile_pool(name="qkp", bufs=2) as qkp,
            tc.tile_pool(name="stgp", bufs=4) as stgp,
            tc.tile_pool(name="vtp", bufs=4) as vtp,
            tc.tile_pool(name="vp", bufs=4) as vp,
            tc.tile_pool(name="ptp", bufs=3) as ptp,
            tc.tile_pool(name="otp", bufs=4) as otp,
            tc.tile_pool(name="outp", bufs=6) as outp,
            tc.tile_pool(name="rcp", bufs=6) as rcp,
            tc.tile_pool(name="psA", bufs=2, space="PSUM") as psA,   # pst [128,1024] x2
            tc.tile_pool(name="psO", bufs=2, space="PSUM") as psO,   # po [66,512] x2
            tc.tile_pool(name="psM", bufs=2, space="PSUM") as psM,   # misc [128,512] x2
        ):
            wq_sb = consts.tile([128, NCH, DH], BF16)
            wk_sb = consts.tile([128, NCH, DH], BF16)
            wv_sb = consts.tile([128, NCH, DH], BF16)
            wo_sb = consts.tile([DH + 2, D + 2], BF16)
            bq_sb = consts.tile([DH, 1], F32)
            bk_sb = consts.tile([DH, 1], F32)
            ident = consts.tile([128, 128], BF16)
            nc.sync.dma_start(out=wq_sb[:], in_=wq.rearrange("(c p) m -> p c m", p=128))
            nc.sync.dma_start(out=wk_sb[:], in_=wk.rearrange("(c p) m -> p c m", p=128))
            nc.sync.dma_start(out=wv_sb[:], in_=wv.rearrange("(c p) m -> p c m", p=128))
            nc.sync.dma_start(out=wo_sb[:], in_=wo_aug[:])
            nc.sync.dma_start(out=bq_sb[:], in_=bq[:])
            nc.sync.dma_start(out=bk_sb[:], in_=bk[:])
            nc.sync.dma_start(out=ident[:], in_=idin[:])

            state = {}

            def alloc_pair(pr):
                st = {"xt": {}, "vt": {}, "v": {}, "ot": {}}
                st["qt"] = qkp.tile([128, S], BF16, tag="qt", name=f"qt_{pr}")
                st["kt"] = qkp.tile([128, S], BF16, tag="kt", name=f"kt_{pr}")
                for half in range(2):
                    b = pr * 2 + half
                    st["vt"][half] = vtp.tile([DH, S], BF16, tag="vt", name=f"vt_{b}")
                state[pr] = st

            def emit_xt_loads(pr):
                st = state[pr]
                for half in range(2):
                    b = pr * 2 + half
                    xts = []
                    for ci in range(NCH):
                        xt_c = xtp.tile([128, S], BF16, tag="xt", name=f"xt_{b}_{ci}")
                        xts.append(xt_c)
                    for blk in range(NQB):
                        for ci in range(NCH):
                            nc.gpsimd.dma_start(
                                out=xts[ci][:, bass.ts(blk, 512)],
                                in_=xT[b, bass.ts(ci, 128), bass.ts(blk, 512)],
                            )
                    st["xt"][half] = xts

            def emit_prep_q(pr, blk, half):
                st = state[pr]
                sl = bass.ts(blk, 512)
                b = pr * 2 + half
                xt = st["xt"][half]
                pq = psM.tile([DH, 512], F32, tag="psM", name=f"pq_{b}_{blk}")
                for ci in range(NCH):
                    nc.tensor.matmul(
                        pq[:], wq_sb[:, ci, :], xt[ci][:, sl],
                        start=(ci == 0), stop=(ci == NCH - 1),
                    )
                if half == 0:
                    nc.vector.tensor_scalar_add(st["qt"][0:DH, sl], pq[:], bq_sb[:])
                else:
                    sq = stgp.tile([DH, 512], BF16, tag="stg", name=f"sq_{b}_{blk}")
                    nc.vector.tensor_scalar_add(sq[:], pq[:], bq_sb[:])
                    nc.sync.dma_start(out=st["qt"][DH:128, sl], in_=sq[:])

            def emit_prep_k(pr, blk, half):
                st = state[pr]
                sl = bass.ts(blk, 512)
                b = pr * 2 + half
                xt = st["xt"][half]
                pk = psM.tile([DH, 512], F32, tag="psM", name=f"pk_{b}_{blk}")
                for ci in range(NCH):
                    nc.tensor.matmul(
                        pk[:], wk_sb[:, ci, :], xt[ci][:, sl],
                        start=(ci == 0), stop=(ci == NCH - 1),
                    )
                if half == 0:
                    nc.vector.tensor_scalar_add(st["kt"][0:DH, sl], pk[:], bk_sb[:])
                else:
                    sk = stgp.tile([DH, 512], BF16, tag="stg", name=f"sk_{b}_{blk}")
                    nc.vector.tensor_scalar_add(sk[:], pk[:], bk_sb[:])
                    nc.sync.dma_start(out=st["kt"][DH:128, sl], in_=sk[:])

            def emit_prep_v(pr, blk, half):
                st = state[pr]
                sl = bass.ts(blk, 512)
                b = pr * 2 + half
                xt = st["xt"][half]
                pvt = psM.tile([DH, 512], F32, tag="psM", name=f"pvt_{b}_{blk}")
                for ci in range(NCH):
                    nc.tensor.matmul(
                        pvt[:], wv_sb[:, ci, :], xt[ci][:, sl],
                        start=(ci == 0), stop=(ci == NCH - 1),
                    )
                nc.vector.tensor_copy(st["vt"][half][:, sl], pvt[:])

            def emit_vtr(pr, vh, half):
                st = state[pr]
                b = pr * 2 + half
                if vh == 0:
                    v_b = vp.tile([128, NKT, DH + 2], BF16, tag="v", name=f"v_{b}")
                    nc.gpsimd.dma_start(out=v_b[:, :, DH:DH + 2], in_=onesin[:])
                    st["v"][half] = v_b
                v_b = st["v"][half]
                pvtr = psM.tile([128, 512], BF16, tag="psM", name=f"pvtr_{b}_{vh}")
                for j in range(8):
                    nc.tensor.transpose(
                        pvtr[:, bass.ts(j, 64)],
                        st["vt"][half][:, bass.ts(vh * 8 + j, 128)],
                        ident[0:DH, 0:DH],
                    )
                nc.vector.tensor_copy(
                    v_b[:, bass.ds(vh * 8, 8), 0:DH],
                    pvtr[:].rearrange("p (k m) -> p k m", m=64),
                )

            def emit_attn_qq(pr, qq, fillers=None, n_fill=1):
                st = state[pr]
                with nc.named_scope(f"attn_{pr}_{qq}"):
                    sl_q = bass.ts(qq, 512)
                    if qq == 0:
                        for half in range(2):
                            st["ot"][half] = otp.tile(
                                [DH + 2, S], BF16, tag="ot", name=f"ot_{pr * 2 + half}"
                            )
                    po = [
                        psO.tile([DH + 2, 512], F32, tag="psO", name=f"po{hb}_{pr}_{qq}")
                        for hb in range(2)
                    ]
                    for kt_i in range(NKT):
                        kt_sl = bass.ts(kt_i, 128)
                        pst = psA.tile([128, 1024], F32, tag="psA", name=f"pst_{pr}_{qq}_{kt_i}")
                        for hb in range(2):
                            nc.tensor.matmul(
                                pst[:, bass.ts(hb, 512)],
                                st["kt"][hb * DH:(hb + 1) * DH, kt_sl],
                                st["qt"][hb * DH:(hb + 1) * DH, sl_q],
                                start=True, stop=True,
                                tile_position=(hb * DH, 0),
                            )
                        ptt = ptp.tile([128, 1024], BF16, tag="pt", name=f"ptt_{pr}_{qq}_{kt_i}")
                        nc.scalar.activation(ptt[:], pst[:], AF.Exp, scale=0.125)
                        for hb in range(2):
                            nc.tensor.matmul(
                                po[hb][:],
                                st["v"][hb][:, kt_i, :],
                                ptt[:, bass.ts(hb, 512)],
                                start=(kt_i == 0), stop=(kt_i == NKT - 1),
                            )
                        if fillers is not None and kt_i % 2 == 1:
                            for _ in range(n_fill):
                                if fillers:
                                    fillers.pop(0)()
                    for hb in range(2):
                        nc.vector.tensor_copy(st["ot"][hb][:, sl_q], po[hb][:])

            def emit_op_tt(pr, half, tt):
                st = state[pr]
                b = pr * 2 + half
                ot_b = st["ot"][half]
                pop = psM.tile([128, 512], F32, tag="psM", name=f"pop_{b}_{tt}")
                pos = psM.tile([128, 2], F32, tag="psM", name=f"pos_{b}_{tt}")
                otc = ot_b[:, bass.ts(tt, 128)]
                nc.tensor.matmul(pop[:], otc, wo_sb[:, 0:D], start=True, stop=True)
                nc.tensor.matmul(pos[:], otc, wo_sb[:, D:D + 2], start=True, stop=True)
                rc = rcp.tile([128, 1], F32, tag="rc", name=f"rc_{b}_{tt}")
                nc.vector.reciprocal(rc[:], pos[:, 0:1])
                so = outp.tile([128, 512], F32, tag="so", name=f"so_{b}_{tt}")
                nc.vector.tensor_scalar_mul(so[:], pop[:], rc[:])
                nc.sync.dma_start(
                    out=out[bass.ds(b * S + tt * 128, 128), :], in_=so[:]
                )

            # ---------------- emission schedule ----------------
            import functools
            alloc_pair(0)
            alloc_pair(1)
            emit_xt_loads(0)
            emit_xt_loads(1)
            # pair-0 prep head (full)
            for blk in range(NQB):
                for half in range(2):
                    emit_prep_q(0, blk, half)
                    emit_prep_k(0, blk, half)
                    emit_prep_v(0, blk, half)
                if blk in (1, 3):
                    for half in range(2):
                        emit_vtr(0, blk // 2, half)

            P = functools.partial
            fill0 = []   # consumed during attn(0, *): all of prep1
            for blk in range(NQB):
                for half in range(2):
                    fill0.append(P(emit_prep_q, 1, blk, half))
                    fill0.append(P(emit_prep_k, 1, blk, half))
                    fill0.append(P(emit_prep_v, 1, blk, half))
                if blk in (1, 3):
                    for half in range(2):
                        fill0.append(P(emit_vtr, 1, blk // 2, half))

            for qq in range(NQB):
                emit_attn_qq(0, qq, fillers=fill0)
            while fill0:
                fill0.pop(0)()

            fill1 = []   # consumed during attn(1, *): both out-projections
            for half in range(2):
                for tt in range(NKT):
                    fill1.append(P(emit_op_tt, 0, half, tt))
            # pair-1 OP pieces interleave after their q block is evacuated:
            # qq0 slots get op0; op1(qq) emitted during attn(1, qq+1)
            for qq in range(NQB):
                emit_attn_qq(1, qq, fillers=fill1, n_fill=2)
                if qq >= 1:
                    for half in range(2):
                        for tt in range((qq - 1) * 4, qq * 4):
                            fill1.append(P(emit_op_tt, 1, half, tt))
            while fill1:
                fill1.pop(0)()
            for half in range(2):
                for tt in range(12, 16):
                    emit_op_tt(1, half, tt)

    nc.compile()
    return nc


def kernel(x, Wq, bq, Wk, bk, Wv, bv, Wo, bo):
    import ml_dtypes
    x = np.asarray(x, dtype=np.float32)
    xT = np.ascontiguousarray(np.transpose(x, (0, 2, 1))).astype(ml_dtypes.bfloat16)
    Wq = np.asarray(Wq, dtype=np.float32)
    Wk = np.asarray(Wk, dtype=np.float32)
    Wv = np.asarray(Wv, dtype=np.float32)
    Wo = np.asarray(Wo, dtype=np.float32)
    bq = np.asarray(bq, dtype=np.float32)
    bk = np.asarray(bk, dtype=np.float32)
    bv = np.asarray(bv, dtype=np.float32)
    bo = np.asarray(bo, dtype=np.float32)

    if "nc" not in _NC_CACHE:
        _NC_CACHE["nc"] = build_kernel()
    nc = _NC_CACHE["nc"]

    eye = np.eye(128).astype(ml_dtypes.bfloat16)
    ones = np.zeros((128, 16, 2), dtype=ml_dtypes.bfloat16)
    ones[:, :, 0] = 1.0
    in_maps = []
    for c in range(NCORES):
        hs = slice(c * DH, (c + 1) * DH)
        wo_a = np.zeros((DH + 2, D + 2), dtype=ml_dtypes.bfloat16)
        wo_a[0:DH, 0:D] = Wo[hs, :]
        wo_a[DH, D] = 1.0
        in_maps.append({
            "xT": xT,
            "wq": np.ascontiguousarray(Wq[:, hs]).astype(ml_dtypes.bfloat16),
            "wk": np.ascontiguousarray(Wk[:, hs]).astype(ml_dtypes.bfloat16),
            "wv": np.ascontiguousarray(Wv[:, hs]).astype(ml_dtypes.bfloat16),
            "wo_aug": wo_a,
            "bq": np.ascontiguousarray(bq[hs].reshape(DH, 1)),
            "bk": np.ascontiguousarray(bk[hs].reshape(DH, 1)),
            "idin": eye,
            "onesin": ones,
        })

    res = run_bass_kernel_spmd(nc, in_maps, list(range(NCORES)))

    acc = np.zeros((B * S, D), dtype=np.float32)
    for c in range(NCORES):
        acc += res.results[c]["out"]
    # biases that commute with the head-reduction, applied at gather time
    acc += bo[None, :] + (bv @ Wo)[None, :]
    return acc.reshape(B, S, D)


# revision 26
# speedup vs baseline: 1.2346x; 1.2346x over previous
"""Trainium2 Bass kernel for nn_MultiHeadAttention (B=4, S=2048, D=512, H=8).

Sharding: tensor-parallel over heads — core c owns head c (Dh=64).
Each core computes q/k/v projections for its head slice (full x replicated,
host-pre-transposed to x^T in bf16), attention for its head over all 4
batches, and the partial out-projection O_c @ Wo[c]; the host sums the 8
partials (the TP all-reduce done at gather time) and adds the biases that
commute with that reduction (bo, bv@Wo).

Engine plan (engines execute their programs in order, so emission order IS
the software pipeline):
  - PE: projections (W-stationary bf16), V^T->V PE-transposes, row-tiled
    S^T pairs (dh=64 contraction on 64x128 array halves), AV with a ones
    column (softmax denominators ride in O^T_aug row 64), out-projection.
  - ACT: exclusively exp(S/8) — it is the pacing engine (~143us floor);
    anything else queued on ACT delays the next attention phase.
  - DVE: all PSUM evacuations + the final normalize (reciprocal of the
    denominator extracted per token partition by an e-column matmul).
  - GPSIMD: bulk x^T loads;  SP: staging shifts + output stores.
Batches are paired [even; odd] on SBUF partition halves; prep of pair 1 is
emitted interleaved with attention of pair 0, and both out-projections are
emitted interleaved with attention of pair 1.
"""
import numpy as np

import concourse.bass as bass
import concourse.mybir as mybir
import concourse.tile as tile
from concourse import bacc
from concourse.bass_utils import run_bass_kernel_spmd

B, S, D = 4, 2048, 512
H, DH = 8, 64
NCORES = 8
F32 = mybir.dt.float32
F32R = mybir.dt.float32r
BF16 = mybir.dt.bfloat16
AF = mybir.ActivationFunctionType

NKT = S // 128          # 16 token tiles per batch
NQB = S // 512          # 4 512-blocks per batch
NCH = D // 128          # 4 dm chunks

_NC_CACHE = {}


def build_kernel():
    nc = bacc.Bacc("TRN2", target_bir_lowering=False, debug=False)

    xT = nc.dram_tensor("xT", [B, D, S], BF16, kind="ExternalInput")
    wq = nc.dram_tensor("wq", [D, DH], BF16, kind="ExternalInput")
    wk = nc.dram_tensor("wk", [D, DH], BF16, kind="ExternalInput")
    wv = nc.dram_tensor("wv", [D, DH], BF16, kind="ExternalInput")
    wo_aug = nc.dram_tensor("wo_aug", [DH + 2, D + 2], BF16, kind="ExternalInput")
    bq = nc.dram_tensor("bq", [DH, 1], F32, kind="ExternalInput")
    bk = nc.dram_tensor("bk", [DH, 1], F32, kind="ExternalInput")
    idin = nc.dram_tensor("idin", [128, 128], BF16, kind="ExternalInput")
    onesin = nc.dram_tensor("onesin", [128, 16, 2], BF16, kind="ExternalInput")
    out = nc.dram_tensor("out", [B * S, D], F32, kind="ExternalOutput")

    with tile.TileContext(nc) as tc:
        with (
            tc.tile_pool(name="consts", bufs=1) as consts,
            tc.tile_pool(name="xtp", bufs=16) as xtp,
            tc.tile_pool(name="qkp", bufs=2) as qkp,
            tc.tile_pool(name="stgp", bufs=4) as stgp,
            tc.tile_pool(name="vtp", bufs=4) as vtp,
            tc.tile_pool(name="vp", bufs=4) as vp,
            tc.tile_pool(name="ptp", bufs=3) as ptp,
            tc.tile_pool(name="otp", bufs=4) as otp,
            tc.tile_pool(name="outp", bufs=6) as outp,
            tc.tile_pool(name="rcp", bufs=6) as rcp,
            tc.tile_pool(name="psA", bufs=2, space="PSUM") as psA,   # pst [128,1024] x2
            tc.tile_pool(name="psO", bufs=2, space="PSUM") as psO,   # po [66,512] x2
            tc.tile_pool(name="psM", bufs=2, space="PSUM") as psM,   # misc [128,512] x2
        ):
            wq_sb = consts.tile([128, NCH, DH], BF16)
            wk_sb = consts.tile([128, NCH, DH], BF16)
            wv_sb = consts.tile([128, NCH, DH], BF16)
            wo_sb = consts.tile([DH + 2, D + 2], BF16)
            bq_sb = consts.tile([DH, 1], F32)
            bk_sb = consts.tile([DH, 1], F32)
            ident = consts.tile([128, 128], BF16)
            nc.sync.dma_start(out=wq_sb[:], in_=wq.rearrange("(c p) m -> p c m", p=128))
            nc.sync.dma_start(out=wk_sb[:], in_=wk.rearrange("(c p) m -> p c m", p=128))
            nc.sync.dma_start(out=wv_sb[:], in_=wv.rearrange("(c p) m -> p c m", p=128))
            nc.sync.dma_start(out=wo_sb[:], in_=wo_aug[:])
            nc.sync.dma_start(out=bq_sb[:], in_=bq[:])
            nc.sync.dma_start(out=bk_sb[:], in_=bk[:])
            nc.sync.dma_start(out=ident[:], in_=idin[:])

            state = {}

            def alloc_pair(pr):
                st = {"xt": {}, "vt": {}, "v": {}, "ot": {}}
                st["qt"] = qkp.tile([128, S], BF16, tag="qt", name=f"qt_{pr}")
                st["kt"] = qkp.tile([128, S], BF16, tag="kt", name=f"kt_{pr}")
                for half in range(2):
                    b = pr * 2 + half
                    st["vt"][half] = vtp.tile([DH, S], BF16, tag="vt", name=f"vt_{b}")
                state[pr] = st

            def emit_xt_loads(pr):
                st = state[pr]
                for half in range(2):
                    b = pr * 2 + half
                    xts = []
                    for ci in range(NCH):
                        xt_c = xtp.tile([128, S], BF16, tag="xt", name=f"xt_{b}_{ci}")
                        xts.append(xt_c)
                    for blk in range(NQB):
                        for ci in range(NCH):
                            nc.gpsimd.dma_start(
                                out=xts[ci][:, bass.ts(blk, 512)],
                                in_=xT[b, bass.ts(ci, 128), bass.ts(blk, 512)],
                            )
                    st["xt"][half] = xts

            def emit_prep_qk(pr, blk, half):
                st = state[pr]
                sl = bass.ts(blk, 512)
                b = pr * 2 + half
                xt = st["xt"][half]
                pqk = psM.tile([128, 512], F32, tag="psM", name=f"pqk_{b}_{blk}")
                pq = pqk[0:DH, :]
                pk = pqk[DH:128, :]
                for ci in range(NCH):
                    nc.tensor.matmul(
                        pq, wq_sb[:, ci, :], xt[ci][:, sl],
                        start=(ci == 0), stop=(ci == NCH - 1),
                    )
                for ci in range(NCH):
                    nc.tensor.matmul(
                        pk, wk_sb[:, ci, :], xt[ci][:, sl],
                        start=(ci == 0), stop=(ci == NCH - 1),
                    )
                if half == 0:
                    nc.vector.tensor_scalar_add(st["qt"][0:DH, sl], pq, bq_sb[:])
                    nc.vector.tensor_scalar_add(st["kt"][0:DH, sl], pk, bk_sb[:])
                else:
                    sq = stgp.tile([DH, 512], BF16, tag="stg", name=f"sq_{b}_{blk}")
                    sk = stgp.tile([DH, 512], BF16, tag="stg", name=f"sk_{b}_{blk}")
                    nc.vector.tensor_scalar_add(sq[:], pq, bq_sb[:])
                    nc.vector.tensor_scalar_add(sk[:], pk, bk_sb[:])
                    nc.sync.dma_start(out=st["qt"][DH:128, sl], in_=sq[:])
                    nc.sync.dma_start(out=st["kt"][DH:128, sl], in_=sk[:])

            def emit_prep_v(pr, blk, half):
                st = state[pr]
                sl = bass.ts(blk, 512)
                b = pr * 2 + half
                xt = st["xt"][half]
                pvt = psM.tile([DH, 512], F32, tag="psM", name=f"pvt_{b}_{blk}")
                for ci in range(NCH):
                    nc.tensor.matmul(
                        pvt[:], wv_sb[:, ci, :], xt[ci][:, sl],
                        start=(ci == 0), stop=(ci == NCH - 1),
                    )
                nc.vector.tensor_copy(st["vt"][half][:, sl], pvt[:])

            def emit_vtr(pr, vh, half):
                st = state[pr]
                b = pr * 2 + half
                if vh == 0:
                    v_b = vp.tile([128, NKT, DH + 2], BF16, tag="v", name=f"v_{b}")
                    nc.gpsimd.dma_start(out=v_b[:, :, DH:DH + 2], in_=onesin[:])
                    st["v"][half] = v_b
                v_b = st["v"][half]
                pvtr = psM.tile([128, 512], BF16, tag="psM", name=f"pvtr_{b}_{vh}")
                for j in range(8):
                    nc.tensor.transpose(
                        pvtr[:, bass.ts(j, 64)],
                        st["vt"][half][:, bass.ts(vh * 8 + j, 128)],
                        ident[0:DH, 0:DH],
                    )
                nc.vector.tensor_copy(
                    v_b[:, bass.ds(vh * 8, 8), 0:DH],
                    pvtr[:].rearrange("p (k m) -> p k m", m=64),
                )

            def emit_attn_qq(pr, qq, fillers=None, n_fill=1):
                st = state[pr]
                with nc.named_scope(f"attn_{pr}_{qq}"):
                    sl_q = bass.ts(qq, 512)
                    if qq == 0:
                        for half in range(2):
                            st["ot"][half] = otp.tile(
                                [DH + 2, S], BF16, tag="ot", name=f"ot_{pr * 2 + half}"
                            )
                    po = [
                        psO.tile([DH + 2, 512], F32, tag="psO", name=f"po{hb}_{pr}_{qq}")
                        for hb in range(2)
                    ]
                    for kt_i in range(NKT):
                        kt_sl = bass.ts(kt_i, 128)
                        pst = psA.tile([128, 1024], F32, tag="psA", name=f"pst_{pr}_{qq}_{kt_i}")
                        for hb in range(2):
                            nc.tensor.matmul(
                                pst[:, bass.ts(hb, 512)],
                                st["kt"][hb * DH:(hb + 1) * DH, kt_sl],
                                st["qt"][hb * DH:(hb + 1) * DH, sl_q],
                                start=True, stop=True,
                                tile_position=(hb * DH, 0),
                            )
                        ptt = ptp.tile([128, 1024], BF16, tag="pt", name=f"ptt_{pr}_{qq}_{kt_i}")
                        nc.scalar.activation(ptt[:], pst[:], AF.Exp, scale=0.125)
                        for hb in range(2):
                            nc.tensor.matmul(
                                po[hb][:],
                                st["v"][hb][:, kt_i, :],
                                ptt[:, bass.ts(hb, 512)],
                                start=(kt_i == 0), stop=(kt_i == NKT - 1),
                            )
                        if fillers is not None and kt_i % 2 == 1:
                            for _ in range(n_fill):
                                if fillers:
                                    fillers.pop(0)()
                    for hb in range(2):
                        nc.vector.tensor_copy(st["ot"][hb][:, sl_q], po[hb][:])

            def emit_op_tt(pr, half, tt):
                st = state[pr]
                b = pr * 2 + half
                ot_b = st["ot"][half]
                pop = psM.tile([128, 512], F32, tag="psM", name=f"pop_{b}_{tt}")
                pos = psM.tile([128, 2], F32, tag="psM", name=f"pos_{b}_{tt}")
                otc = ot_b[:, bass.ts(tt, 128)]
                nc.tensor.matmul(pop[:], otc, wo_sb[:, 0:D], start=True, stop=True)
                nc.tensor.matmul(pos[:], otc, wo_sb[:, D:D + 2], start=True, stop=True)
                rc = rcp.tile([128, 1], F32, tag="rc", name=f"rc_{b}_{tt}")
                nc.vector.reciprocal(rc[:], pos[:, 0:1])
                so = outp.tile([128, 512], F32, tag="so", name=f"so_{b}_{tt}")
                nc.vector.tensor_scalar_mul(so[:], pop[:], rc[:])
                nc.sync.dma_start(
                    out=out[bass.ds(b * S + tt * 128, 128), :], in_=so[:]
                )

            # ---------------- emission schedule ----------------
            import functools
            alloc_pair(0)
            alloc_pair(1)
            emit_xt_loads(0)
            emit_xt_loads(1)
            # pair-0 prep head (full)
            for blk in range(NQB):
                for half in range(2):
                    emit_prep_qk(0, blk, half)
                    emit_prep_v(0, blk, half)
                if blk in (1, 3):
                    for half in range(2):
                        emit_vtr(0, blk // 2, half)

            P = functools.partial
            fill0 = []   # consumed during attn(0, *): all of prep1
            for blk in range(NQB):
                for half in range(2):
                    fill0.append(P(emit_prep_qk, 1, blk, half))
                    fill0.append(P(emit_prep_v, 1, blk, half))
                if blk in (1, 3):
                    for half in range(2):
                        fill0.append(P(emit_vtr, 1, blk // 2, half))

            for qq in range(NQB):
                emit_attn_qq(0, qq, fillers=fill0)
            while fill0:
                fill0.pop(0)()

            fill1 = []   # consumed during attn(1, *): both out-projections
            for half in range(2):
                for tt in range(NKT):
                    fill1.append(P(emit_op_tt, 0, half, tt))
            # pair-1 OP pieces interleave after their q block is evacuated:
            # qq0 slots get op0; op1(qq) emitted during attn(1, qq+1)
            for qq in range(NQB):
                emit_attn_qq(1, qq, fillers=fill1, n_fill=2)
                if qq >= 1:
                    for half in range(2):
                        for tt in range((qq - 1) * 4, qq * 4):
                            fill1.append(P(emit_op_tt, 1, half, tt))
            while fill1:
                fill1.pop(0)()
            for half in range(2):
                for tt in range(12, 16):
                    emit_op_tt(1, half, tt)

    nc.compile()
    return nc


def kernel(x, Wq, bq, Wk, bk, Wv, bv, Wo, bo):
    import ml_dtypes
    x = np.asarray(x, dtype=np.float32)
    xT = np.ascontiguousarray(np.transpose(x, (0, 2, 1))).astype(ml_dtypes.bfloat16)
    Wq = np.asarray(Wq, dtype=np.float32)
    Wk = np.asarray(Wk, dtype=np.float32)
    Wv = np.asarray(Wv, dtype=np.float32)
    Wo = np.asarray(Wo, dtype=np.float32)
    bq = np.asarray(bq, dtype=np.float32)
    bk = np.asarray(bk, dtype=np.float32)
    bv = np.asarray(bv, dtype=np.float32)
    bo = np.asarray(bo, dtype=np.float32)

    if "nc" not in _NC_CACHE:
        _NC_CACHE["nc"] = build_kernel()
    nc = _NC_CACHE["nc"]

    eye = np.eye(128).astype(ml_dtypes.bfloat16)
    ones = np.zeros((128, 16, 2), dtype=ml_dtypes.bfloat16)
    ones[:, :, 0] = 1.0
    in_maps = []
    for c in range(NCORES):
        hs = slice(c * DH, (c + 1) * DH)
        wo_a = np.zeros((DH + 2, D + 2), dtype=ml_dtypes.bfloat16)
        wo_a[0:DH, 0:D] = Wo[hs, :]
        wo_a[DH, D] = 1.0
        in_maps.append({
            "xT": xT,
            "wq": np.ascontiguousarray(Wq[:, hs]).astype(ml_dtypes.bfloat16),
            "wk": np.ascontiguousarray(Wk[:, hs]).astype(ml_dtypes.bfloat16),
            "wv": np.ascontiguousarray(Wv[:, hs]).astype(ml_dtypes.bfloat16),
            "wo_aug": wo_a,
            "bq": np.ascontiguousarray(bq[hs].reshape(DH, 1)),
            "bk": np.ascontiguousarray(bk[hs].reshape(DH, 1)),
            "idin": eye,
            "onesin": ones,
        })

    res = run_bass_kernel_spmd(nc, in_maps, list(range(NCORES)))

    acc = np.zeros((B * S, D), dtype=np.float32)
    for c in range(NCORES):
        acc += res.results[c]["out"]
    # biases that commute with the head-reduction, applied at gather time
    acc += bo[None, :] + (bv @ Wo)[None, :]
    return acc.reshape(B, S, D)
